# revision 1
# baseline (speedup 1.0000x reference)
"""Trainium2 Bass kernel for nn_DensityVQC (batched 2-qubit VQC Z-expectation).

Algebra
-------
The reference builds rho_b = conj(psi_b) psi_b^T (note: transpose of the
standard density matrix), evolves rho' = U rho U^dag and returns
tr(rho' Z0) with Z0 = diag(1,1,-1,-1).  This collapses to a per-row
quadratic form: with V = conj(U) (the transposed-rho convention flips the
conjugation) and phi = V psi,

    out_b = |phi_0|^2 + |phi_1|^2 - |phi_2|^2 - |phi_3|^2
          = 2 * || C psi_b ||^2 - ||psi_b||^2        (C = V[0:2, :], U unitary)
          = || A r_b + B m_b ||^2 - 1                (inputs are unit-norm)

with real 4x4 matrices A = sqrt(2)*[Re C; Im C], B = sqrt(2)*[-Im C; Re C].
So the device kernel is: per batch row (r, m in R^4), compute w = A r + B m,
then out = sum(w^2) - 1.  No [B,4,4] density matrices are ever materialized.

Device mapping (per core, pure data parallel over 8 cores)
----------------------------------------------------------
Host-side marshalling (the sharding step) reshapes each core's slice into
component-major layout [128 = 32 groups x 4 comps, 4096] so the device
needs no transposes; loads are perfectly contiguous plain DMAs.

Per supertile of 512 free columns (16384 batch rows):
  1. PE: phi = blkdiag32(A^T)^T . rt + blkdiag32(B^T)^T . mt  (two
     accumulating float32r matmuls at full PE rate, moving operands are
     DMA-resident input slices)
  2. ACT Square: S = phi^2 -> SBUF (f32r)
  3. PE: one reduce matmul (stationary = group-sum pattern [128,32],
     moving = S) -> out32 [32, 512] in PSUM
  4. ACT/DVE copy with -1 bias -> resident [32, 4096] output tile
A dummy-matmul burst during the load window warms the PE HAM clock-gate so
the real matmuls run at 2.4 GHz.  The host un-permutes the [32, 4096]
output tile back to batch order (pure data marshalling).
"""

import sys
import numpy as np

if "/opt/trn_rl_repo" not in sys.path:
    sys.path.insert(0, "/opt/trn_rl_repo")

import concourse.bass as bass
import concourse.tile as tile
from concourse import bacc, mybir
from concourse import bass_utils
from concourse.tile_rust import add_dep_helper

N_CORES = 8
BSZ = 1_048_576
BC = BSZ // N_CORES            # 131072 rows per core
NCOL = BC // 32                # 4096 component-major free columns
N_ST = NCOL // 512             # 8 supertiles
N_WARM = 0                     # HAM warm-up (disabled: loads are the bottleneck and PE re-throttles between chunks anyway)
F32 = mybir.dt.float32
F32R = mybir.dt.float32r
N_LAYERS = 6


def _circuit_unitary(ry, rz):
    """4x4 circuit unitary, float64 mirror of reference._circuit_unitary."""
    ry = np.asarray(ry, dtype=np.float64)
    rz = np.asarray(rz, dtype=np.float64)
    cnot = np.array(
        [[1, 0, 0, 0], [0, 1, 0, 0], [0, 0, 0, 1], [0, 0, 1, 0]],
        dtype=np.complex128,
    )

    def _ry(th):
        c, s = np.cos(th / 2), np.sin(th / 2)
        return np.array([[c, -s], [s, c]], dtype=np.complex128)

    def _rz(th):
        return np.diag([np.exp(-0.5j * th), np.exp(0.5j * th)])

    u = np.eye(4, dtype=np.complex128)
    for l in range(ry.shape[0]):
        ry_full = np.kron(_ry(ry[l, 0]), _ry(ry[l, 1]))
        rz_full = np.kron(_rz(rz[l, 0]), _rz(rz[l, 1]))
        u = cnot @ (rz_full @ (ry_full @ u))
    return u


def _host_consts(ry_params, rz_params):
    u = _circuit_unitary(ry_params, rz_params)
    c = np.conj(u)[0:2, :]
    a = np.sqrt(2.0) * np.vstack([c.real, c.imag])     # 4x4, w = A r + B m
    b = np.sqrt(2.0) * np.vstack([-c.imag, c.real])
    eye32 = np.eye(32, dtype=np.float32)
    # lhsT[k=4g+c, m=4g+j] = A[j, c]  ->  block_diag of A.T
    ablk = np.kron(eye32, a.T.astype(np.float32)).astype(np.float32)
    bblk = np.kron(eye32, b.T.astype(np.float32)).astype(np.float32)
    zsum = np.kron(eye32, np.ones((4, 1), dtype=np.float32)).astype(np.float32)
    # Four partition-shifted reduce patterns: zq[k, 32q+g] = zsum[k, g].
    # Supertile st (q = st%4) accumulates its group-sums into partitions
    # [32q, 32q+32) of a shared full-width PSUM bank.
    zqs = []
    for q in range(4):
        zq = np.zeros((128, 128), dtype=np.float32)
        zq[:, 32 * q : 32 * (q + 1)] = zsum
        zqs.append(zq)
    return ablk, bblk, zqs


# Any fixed permutation of the 4096 32-row blocks works (the host inverts
# it); identity keeps the input marshalling a pure reshape+transpose.
def _to_component_major(x):
    """x [BC,4] f32 -> [128, NCOL] f32: column N holds batch rows
    [32N, 32N+32) x 4 comps on the 128 partitions."""
    return np.ascontiguousarray(x.reshape(NCOL, 128).T)


def _from_out32(y):
    """y [2, 128, 512] -> [BC]: value for supertile st = 4h+q, col n, group g
    lives at y[h, 32q+g, n]; batch b = 16384*st + 32n + g."""
    return np.ascontiguousarray(
        y.reshape(2, 4, 32, 512).transpose(0, 1, 3, 2)
    ).reshape(-1)


def _build_program():
    nc = bacc.Bacc("TRN2", target_bir_lowering=False, debug=False)
    # Consts ride as leading columns of the input tensors so no separate
    # DMA (descgen + completion receipt) delays the first data chunk.
    rt_d = nc.dram_tensor("rt", [128, 512 + NCOL], F32R, kind="ExternalInput")
    mt_d = nc.dram_tensor("mt", [128, 256 + NCOL], F32R, kind="ExternalInput")
    out_d = nc.dram_tensor("out", [2, 128, 512], F32, kind="ExternalOutput")

    out_lo_d = out_d.ap()[0]
    out_hi_d = out_d.ap()[1]

    with tile.TileContext(nc) as tc:
        with (
            tc.tile_pool(name="const", bufs=1) as cpool,
            tc.tile_pool(name="io", bufs=1) as iopool,
            tc.tile_pool(name="work", bufs=4) as wpool,
            tc.tile_pool(name="psum", bufs=3, space=bass.MemorySpace.PSUM) as ppool,
        ):
            rt_t = iopool.tile([128, 512 + NCOL], F32R, name="rt_t")
            mt_t = iopool.tile([128, 256 + NCOL], F32R, name="mt_t")
            zq = [rt_t[:, 128 * q : 128 * (q + 1)] for q in range(4)]
            ablk = mt_t[:, 0:128]
            bblk = mt_t[:, 128:256]
            half = NCOL // 2
            # Full 128-partition output tiles (a 32-partition tile only uses
            # 1/4 of the SBUF DMA ports): supertile st lands on partitions
            # [32*(st%4), +32) at columns [512*(st//4), +512).
            out_lo = iopool.tile([128, 512], F32, name="out_lo")
            out_hi = iopool.tile([128, 512], F32, name="out_hi")

            # Small first/last data chunks (early start, short tail); the
            # first chunk of each tensor also carries its consts.
            rb = [0, 1024, 2048, 3072, 4096, 512 + NCOL]
            mb = [0, 768, 1792, 2816, 3840, 256 + NCOL]
            prev_r, prev_m = None, None
            for q in range(len(rb) - 1):
                rqs = bass.ds(rb[q], rb[q + 1] - rb[q])
                mqs = bass.ds(mb[q], mb[q + 1] - mb[q])
                r_dma = nc.sync.dma_start(rt_t[:, rqs], rt_d.ap()[:, rqs])
                m_dma = nc.scalar.dma_start(mt_t[:, mqs], mt_d.ap()[:, mqs])
                # Ordering-only edges: keep the scheduler from reordering
                # chunks (queues are FIFO; a late-scheduled early chunk
                # stalls consumers).
                if prev_r is not None:
                    add_dep_helper(r_dma.ins, prev_r.ins, sync=False, reason="q")
                    add_dep_helper(m_dma.ins, prev_m.ins, sync=False, reason="q")
                prev_r, prev_m = r_dma, m_dma

            # HAM warm-up: dense dummy matmuls on the const tile keep the PE
            # busy through the load window so real matmuls run at 2.4 GHz.
            # Two alternating PSUM buffers so warm-up matmuls pipeline
            # back-to-back (a single buffer serializes on fill-after-drain).
            warm_a = ppool.tile([128, 512], F32, name="warm_a", bufs=1)
            warm_b = ppool.tile([128, 512], F32, name="warm_b", bufs=1)
            for w in range(N_WARM):
                nc.tensor.matmul((warm_a if w % 2 else warm_b)[:], ablk, cstz[:])

            for st in range(N_ST):
                cs = bass.ts(st, 512)
                phi = ppool.tile([128, 512], F32, name="phi", bufs=4)
                nc.tensor.matmul(
                    phi[:], ablk, rt_t[:, 512 + 512 * st : 512 + 512 * (st + 1)],
                    start=True, stop=False,
                )
                nc.tensor.matmul(
                    phi[:], bblk, mt_t[:, 256 + 512 * st : 256 + 512 * (st + 1)],
                    start=False, stop=True,
                )

                s_sb = wpool.tile([128, 512], F32R, name="s_sb")
                nc.scalar.activation(
                    s_sb[:], phi[:], mybir.ActivationFunctionType.Square
                )

                q = st % 4
                if q == 0:
                    ored = ppool.tile([128, 512], F32, name="ored", bufs=2)
                nc.tensor.matmul(
                    ored[:], zq[q], s_sb[:], start=(q == 0), stop=(q == 3)
                )

                if q == 3:
                    # One full-width PSUM -> SBUF copy (with the -1 fold)
                    # per half; alternate engines.
                    out_t = out_lo if st < 4 else out_hi
                    if st < 4:
                        nc.scalar.activation(
                            out_t[:],
                            ored[:],
                            mybir.ActivationFunctionType.Copy,
                            bias=-1.0,
                        )
                    else:
                        nc.vector.tensor_scalar_add(out_t[:], ored[:], -1.0)

                if st == 3:
                    nc.sync.dma_start(out_lo_d, out_lo[:])
            nc.sync.dma_start(out_hi_d, out_hi[:])
    nc.compile()
    return nc


_PROG_CACHE = None


def _get_program():
    global _PROG_CACHE
    if _PROG_CACHE is None:
        _PROG_CACHE = _build_program()
    return _PROG_CACHE


def _run(ry_params, rz_params, states_real, states_imag, **hw_kwargs):
    ablk, bblk, zqs = _host_consts(ry_params, rz_params)
    csta = np.concatenate([ablk, bblk], axis=1).astype(np.float32)
    cstz = np.concatenate(zqs, axis=1).astype(np.float32)
    states_real = np.ascontiguousarray(states_real, dtype=np.float32)
    states_imag = np.ascontiguousarray(states_imag, dtype=np.float32)
    in_maps = []
    for k in range(N_CORES):
        sl = slice(k * BC, (k + 1) * BC)
        in_maps.append(
            {
                "rt": np.concatenate(
                    [cstz, _to_component_major(states_real[sl])], axis=1
                ),
                "mt": np.concatenate(
                    [csta, _to_component_major(states_imag[sl])], axis=1
                ),
            }
        )
    nc = _get_program()
    res = bass_utils.run_bass_kernel_spmd(
        nc, in_maps, core_ids=list(range(N_CORES)), **hw_kwargs
    )
    out = np.concatenate(
        [_from_out32(res.results[k]["out"]) for k in range(N_CORES)]
    ).astype(np.float32)
    return out, res


def kernel(ry_params, rz_params, states_real, states_imag):
    out, _ = _run(ry_params, rz_params, states_real, states_imag)
    return out



# revision 4
# speedup vs baseline: 1.2984x; 1.2984x over previous
"""Trainium2 Bass kernel for nn_DensityVQC (batched 2-qubit VQC Z-expectation).

Algebra
-------
The reference builds rho_b = conj(psi_b) psi_b^T (note: transpose of the
standard density matrix), evolves rho' = U rho U^dag and returns
tr(rho' Z0) with Z0 = diag(1,1,-1,-1).  This collapses to a per-row
quadratic form: with V = conj(U) (the transposed-rho convention flips the
conjugation) and phi = V psi,

    out_b = |phi_0|^2 + |phi_1|^2 - |phi_2|^2 - |phi_3|^2
          = 2 * || C psi_b ||^2 - ||psi_b||^2        (C = V[0:2, :], U unitary)
          = || A r_b + B m_b ||^2 - 1                (inputs are unit-norm)

with real 4x4 matrices A = sqrt(2)*[Re C; Im C], B = sqrt(2)*[-Im C; Re C].
So the device kernel is: per batch row (r, m in R^4), compute w = A r + B m,
then out = sum(w^2) - 1.  No [B,4,4] density matrices are ever materialized.

Device mapping (per core, pure data parallel over 8 cores)
----------------------------------------------------------
Everything on the wire and through the PE runs in fp16 (measured end-to-end
rel err ~1e-3, tolerance is 2e-2): it halves HBM traffic vs f32 AND runs
matmuls at 1 col/cycle (f32 "HIGH" mode costs 2-4x on HW).  Host-side
marshalling reshapes each core's slice into component-major layout
[128 = 32 groups x 4 comps, 4096] fp16 so loads are contiguous plain DMAs.

Per supertile PAIR (2 x 512 free columns = 32768 batch rows):
  1. PE: 4 accumulating fp16 matmuls into one [128,1024] 2-bank PSUM tile
     (phi for both supertiles; PSUM accumulates f32)
  2. ONE ACT Square [128,1024] -> fp16 SBUF (squares are ACT-only: DVE
     cannot read two PSUM operands; batching pairs amortizes the fixed
     per-instruction overhead)
  3. PE: two reduce matmuls (stationary = signed group-sum pattern, fp16),
     4 supertiles accumulate into one full-width [128,512] PSUM bank
  4. DVE copy with -1 fold -> resident [128,512] fp16 output tile, DMA out
rt streams on the sync queue, mt on the gpsimd queue (the scalar engine is
kept free for the squares).  The host un-permutes the fp16 output back to
batch order and upcasts to f32.
"""

import sys
import numpy as np

if "/opt/trn_rl_repo" not in sys.path:
    sys.path.insert(0, "/opt/trn_rl_repo")

import concourse.bass as bass
import concourse.tile as tile
from concourse import bacc, mybir
from concourse import bass_utils
from concourse.tile_rust import add_dep_helper

N_CORES = 8
BSZ = 1_048_576
BC = BSZ // N_CORES            # 131072 rows per core
NCOL = BC // 32                # 4096 component-major free columns
N_ST = NCOL // 512             # 8 supertiles
F32 = mybir.dt.float32
F16 = mybir.dt.float16
N_LAYERS = 6


def _circuit_unitary(ry, rz):
    """4x4 circuit unitary, float64 mirror of reference._circuit_unitary."""
    ry = np.asarray(ry, dtype=np.float64)
    rz = np.asarray(rz, dtype=np.float64)
    cnot = np.array(
        [[1, 0, 0, 0], [0, 1, 0, 0], [0, 0, 0, 1], [0, 0, 1, 0]],
        dtype=np.complex128,
    )

    def _ry(th):
        c, s = np.cos(th / 2), np.sin(th / 2)
        return np.array([[c, -s], [s, c]], dtype=np.complex128)

    def _rz(th):
        return np.diag([np.exp(-0.5j * th), np.exp(0.5j * th)])

    u = np.eye(4, dtype=np.complex128)
    for l in range(ry.shape[0]):
        ry_full = np.kron(_ry(ry[l, 0]), _ry(ry[l, 1]))
        rz_full = np.kron(_rz(rz[l, 0]), _rz(rz[l, 1]))
        u = cnot @ (rz_full @ (ry_full @ u))
    return u


def _host_consts(ry_params, rz_params):
    u = _circuit_unitary(ry_params, rz_params)
    c = np.conj(u)[0:2, :]
    a = np.sqrt(2.0) * np.vstack([c.real, c.imag])     # 4x4, w = A r + B m
    b = np.sqrt(2.0) * np.vstack([-c.imag, c.real])
    eye32 = np.eye(32, dtype=np.float32)
    # lhsT[k=4g+c, m=4g+j] = A[j, c]  ->  block_diag of A.T
    ablk = np.kron(eye32, a.T).astype(np.float16)
    bblk = np.kron(eye32, b.T).astype(np.float16)
    zsum = np.kron(eye32, np.ones((4, 1), dtype=np.float32))
    # Four partition-shifted reduce patterns: zq[k, 32q+g] = zsum[k, g].
    # Supertile st (q = st%4) accumulates its group-sums into partitions
    # [32q, 32q+32) of a shared full-width PSUM bank.
    zqs = []
    for q in range(4):
        zq = np.zeros((128, 128), dtype=np.float32)
        zq[:, 32 * q : 32 * (q + 1)] = zsum
        zqs.append(zq.astype(np.float16))
    return ablk, bblk, zqs


# Any fixed permutation of the 4096 32-row blocks works (the host inverts
# it); identity keeps the input marshalling a pure reshape+transpose.
def _to_component_major(x):
    """x [BC,4] f32 -> [128, NCOL] fp16: column N holds batch rows
    [32N, 32N+32) x 4 comps on the 128 partitions."""
    return np.ascontiguousarray(x.reshape(NCOL, 128).T.astype(np.float16))


def _from_out32(y):
    """y [2, 128, 512] -> [BC]: value for supertile st = 4h+q, col n, group g
    lives at y[h, 32q+g, n]; batch b = 16384*st + 32n + g."""
    return np.ascontiguousarray(
        y.astype(np.float32).reshape(2, 4, 32, 512).transpose(0, 1, 3, 2)
    ).reshape(-1)


def _build_program():
    nc = bacc.Bacc("TRN2", target_bir_lowering=False, debug=False)
    # Consts ride as leading columns of the input tensors so no separate
    # DMA (descgen + completion receipt) delays the first data chunk.
    rt_d = nc.dram_tensor("rt", [128, 512 + NCOL], F16, kind="ExternalInput")
    mt_d = nc.dram_tensor("mt", [128, 256 + NCOL], F16, kind="ExternalInput")
    out_d = nc.dram_tensor("out", [2, 128, 512], F16, kind="ExternalOutput")

    out_lo_d = out_d.ap()[0]
    out_hi_d = out_d.ap()[1]

    with tile.TileContext(nc) as tc:
        with (
            tc.tile_pool(name="io", bufs=1) as iopool,
            tc.tile_pool(name="work", bufs=4) as wpool,
            tc.tile_pool(name="psum", bufs=3, space=bass.MemorySpace.PSUM) as ppool,
        ):
            rt_t = iopool.tile([128, 512 + NCOL], F16, name="rt_t")
            mt_t = iopool.tile([128, 256 + NCOL], F16, name="mt_t")
            zq = [rt_t[:, 128 * q : 128 * (q + 1)] for q in range(4)]
            ablk = mt_t[:, 0:128]
            bblk = mt_t[:, 128:256]
            # Full 128-partition output tiles (a 32-partition tile only uses
            # 1/4 of the SBUF DMA ports): supertile st lands on partitions
            # [32*(st%4), +32) at columns [512*(st//4), +512).
            out_lo = iopool.tile([128, 512], F16, name="out_lo")
            out_hi = iopool.tile([128, 512], F16, name="out_hi")

            # Small first/last data chunks (early start, short tail); the
            # first chunk of each tensor also carries its consts.
            rb = [0, 1024, 2048, 3072, 4096, 512 + NCOL]
            mb = [0, 768, 1792, 2816, 3840, 256 + NCOL]
            prev_r, prev_m = None, None
            for q in range(len(rb) - 1):
                rqs = bass.ds(rb[q], rb[q + 1] - rb[q])
                mqs = bass.ds(mb[q], mb[q + 1] - mb[q])
                r_dma = nc.sync.dma_start(rt_t[:, rqs], rt_d.ap()[:, rqs])
                m_dma = nc.gpsimd.dma_start(mt_t[:, mqs], mt_d.ap()[:, mqs])
                # Ordering-only edges: keep the scheduler from reordering
                # chunks (queues are FIFO; a late-scheduled early chunk
                # stalls consumers).
                if prev_r is not None:
                    add_dep_helper(r_dma.ins, prev_r.ins, sync=False, reason="q")
                    add_dep_helper(m_dma.ins, prev_m.ins, sync=False, reason="q")
                prev_r, prev_m = r_dma, m_dma

            for p in range(N_ST // 2):
                # phi for supertiles (2p, 2p+1) side by side in a 2-bank
                # PSUM tile; each 512-col half is its own accumulation
                # group within one bank.
                phi = ppool.tile([128, 1024], F32, name="phi", bufs=3)
                for h in range(2):
                    st = 2 * p + h
                    hs = bass.ds(512 * h, 512)
                    nc.tensor.matmul(
                        phi[:, hs], ablk,
                        rt_t[:, 512 + 512 * st : 512 + 512 * (st + 1)],
                        start=True, stop=False,
                    )
                    nc.tensor.matmul(
                        phi[:, hs], bblk,
                        mt_t[:, 256 + 512 * st : 256 + 512 * (st + 1)],
                        start=False, stop=True,
                    )

                s_sb = wpool.tile([128, 1024], F16, name="s_sb", bufs=3)
                nc.scalar.activation(
                    s_sb[:], phi[:], mybir.ActivationFunctionType.Square
                )

                if p % 2 == 0:
                    ored = ppool.tile([128, 512], F32, name="ored", bufs=2)
                for h in range(2):
                    st = 2 * p + h
                    q = st % 4
                    nc.tensor.matmul(
                        ored[:], zq[q], s_sb[:, bass.ds(512 * h, 512)],
                        start=(q == 0), stop=(q == 3),
                    )

                if p % 2 == 1:
                    # One full-width PSUM -> SBUF copy (with the -1 fold)
                    # per half, on the vector engine (scalar is busy
                    # squaring).
                    out_t = out_lo if p < 2 else out_hi
                    nc.vector.tensor_scalar_add(out_t[:], ored[:], -1.0)

                if p == 1:
                    nc.sync.dma_start(out_lo_d, out_lo[:])
            nc.sync.dma_start(out_hi_d, out_hi[:])
    nc.compile()
    return nc


_PROG_CACHE = None


def _get_program():
    global _PROG_CACHE
    if _PROG_CACHE is None:
        _PROG_CACHE = _build_program()
    return _PROG_CACHE


def _run(ry_params, rz_params, states_real, states_imag, **hw_kwargs):
    ablk, bblk, zqs = _host_consts(ry_params, rz_params)
    csta = np.concatenate([ablk, bblk], axis=1).astype(np.float16)
    cstz = np.concatenate(zqs, axis=1).astype(np.float16)
    states_real = np.ascontiguousarray(states_real, dtype=np.float32)
    states_imag = np.ascontiguousarray(states_imag, dtype=np.float32)
    in_maps = []
    for k in range(N_CORES):
        sl = slice(k * BC, (k + 1) * BC)
        in_maps.append(
            {
                "rt": np.concatenate(
                    [cstz, _to_component_major(states_real[sl])], axis=1
                ),
                "mt": np.concatenate(
                    [csta, _to_component_major(states_imag[sl])], axis=1
                ),
            }
        )
    nc = _get_program()
    res = bass_utils.run_bass_kernel_spmd(
        nc, in_maps, core_ids=list(range(N_CORES)), **hw_kwargs
    )
    out = np.concatenate(
        [_from_out32(res.results[k]["out"]) for k in range(N_CORES)]
    ).astype(np.float32)
    return out, res


def kernel(ry_params, rz_params, states_real, states_imag):
    out, _ = _run(ry_params, rz_params, states_real, states_imag)
    return out
